# revision 36
# baseline (speedup 1.0000x reference)
"""BertSelfAttention with relative inference-path bias — Bass/Tile TRN2 kernel.

Shapes: B=2, S=128, H=12, DH=64, HID=768.  8 NeuronCores.

The reference materializes ip = inference_path @ Wip ([B,S,S,1536], 201MB)
and 5D ra/rb tensors; it reinterprets ra/rb via a RAW flat view
(`.reshape(B,H,S,S,DH)` on a [B,S,S,HID] array — torch .view semantics),
which scrambles cells: score pair w = (h,q') with w = h*128+q' draws its
bias row from raw cells t = 12*k_raw + c_idx of projection block
q_raw = w // 12, where (j, k') = divmod(t, 128), j = w - 12*q_raw.

Sharding: (b, q_raw) blocks, 32 per core; each core covers 3 heads x all
128 queries of its batch and emits a PARTIAL mlp output summed on the
host across the 4 cores of each batch.

v1: fp8(e4m3) DoubleRow projection.  The host pre-transposes + casts ip
to fp8 (so no on-chip transpose) and stores Wip * 16 in fp8 with columns
permuted to (m_hi, half, d) so the PSUM projection comes out with the
a/b halves interleaved at 64-elem granularity.  The DRAM round-trip then
needs ONE scrambled read per block (256B elements instead of 2x128B).
Wq/Wk/bq/bk are scaled x16 to match (scores come out x256; folded into
the softmax exp scale).
"""
import math
import sys

import numpy as np

if '/opt/trn_rl_repo' not in sys.path:
    sys.path.insert(0, '/opt/trn_rl_repo')

H = 12
DH = 64
HID = 768
B = 2
S = 128
N_CORES = 8
QS = (B * S) // N_CORES   # 32 q_raw blocks per core
NH = 3                    # heads per core
HS = NH * DH              # 192
HSP = 256                 # padded head-slice width (fp32r needs N>=256)
WSC = 16.0                # fp8 scale on Wip/Wq/Wk (scores x 16*16)

_CACHE = {}


def _build_program():
    import concourse.bass as bass
    import concourse.bacc as bacc
    import concourse.mybir as mybir
    import concourse.tile as tile

    f32 = mybir.dt.float32
    f32r = mybir.dt.float32r
    bf16 = mybir.dt.bfloat16
    f8 = mybir.dt.float8e4
    rt = bf16                  # round-trip dtype for scrambled tensors
    AX = mybir.AxisListType
    OP = mybir.AluOpType
    ACTF = mybir.ActivationFunctionType
    DR = mybir.MatmulPerfMode.DoubleRow

    nc = bacc.Bacc("TRN2", target_bir_lowering=False, debug=False,
                   num_devices=N_CORES)

    # ---- DRAM I/O (per-core shapes) ----
    ip_d = nc.dram_tensor("ip", [QS, S, HID], f8, kind="ExternalInput").ap()
    hst_d = nc.dram_tensor("hst", [HID, S], f32r, kind="ExternalInput").ap()
    mask_d = nc.dram_tensor("mask", [S, 1], f32, kind="ExternalInput").ap()
    spant_d = nc.dram_tensor("spant", [S, S], f32r, kind="ExternalInput").ap()
    wq_d = nc.dram_tensor("wq", [HID, HSP], f32r, kind="ExternalInput").ap()
    wk_d = nc.dram_tensor("wk", [HID, HSP], f32r, kind="ExternalInput").ap()
    wv_d = nc.dram_tensor("wv", [HID, HSP], f32r, kind="ExternalInput").ap()
    wpv_d = nc.dram_tensor("wpv", [HID, DH], f32r, kind="ExternalInput").ap()
    # wip: [3][128][two=2][2*HID] fp8, columns permuted (m_hi, half, d), x16
    wip_d = nc.dram_tensor("wip", [3, 128, 2, 2 * HID], f8,
                           kind="ExternalInput").ap()
    wma_d = nc.dram_tensor("wmlpa", [128, HID], f32r, kind="ExternalInput").ap()
    wmb_d = nc.dram_tensor("wmlpb", [128, HID], f32r, kind="ExternalInput").ap()
    bq_d = nc.dram_tensor("bq", [1, HSP], f32r, kind="ExternalInput").ap()
    bk_d = nc.dram_tensor("bk", [1, HSP], f32r, kind="ExternalInput").ap()
    bv_d = nc.dram_tensor("bv", [1, HSP], f32r, kind="ExternalInput").ap()
    bpv_d = nc.dram_tensor("bpv", [1, DH], f32r, kind="ExternalInput").ap()
    bmlp_d = nc.dram_tensor("bmlp", [1, HID], f32r, kind="ExternalInput").ap()
    ones_d = nc.dram_tensor("ones", [S, 1], f32, kind="ExternalInput").ap()
    onesr_d = nc.dram_tensor("onesr", [1, S], f32r, kind="ExternalInput").ap()
    out_d = nc.dram_tensor("out", [S, HID], f32, kind="ExternalOutput").ap()

    with tile.TileContext(nc) as tc:
        with (
            tc.tile_pool(name="wpool", bufs=1) as wpool,
            tc.tile_pool(name="cpool", bufs=1) as cpool,
            tc.tile_pool(name="iopool", bufs=4) as iopool,
            tc.tile_pool(name="bpool", bufs=6) as bpool,
            tc.tile_pool(name="mid", bufs=4) as mid,
            tc.tile_pool(name="ppool", bufs=2, space=bass.MemorySpace.PSUM) as ppool,
            tc.tile_pool(name="tpool", bufs=1, space=bass.MemorySpace.PSUM) as tpool,
            tc.tile_pool(name="dpool", bufs=6, space=bass.MemorySpace.DRAM) as dpool,
            tc.tile_pool(name="dpool1", bufs=1, space=bass.MemorySpace.DRAM) as dpool1,
        ):
            # ---- constants / weights to SBUF ----
            ones = cpool.tile([S, 1], f32, tag="ones")
            nc.scalar.dma_start(ones[:], ones_d)
            onesr = cpool.tile([1, S], f32r, tag="onesr")
            nc.scalar.dma_start(onesr[:], onesr_d)
            mask = cpool.tile([S, 1], f32, tag="mask")
            nc.scalar.dma_start(mask[:], mask_d)
            spant = cpool.tile([S, S], f32r, tag="spant")
            nc.scalar.dma_start(spant[:], spant_d)
            hst = cpool.tile([S, HID], f32r, tag="hst")
            for i in range(6):
                nc.gpsimd.dma_start(hst[:, i * 128:(i + 1) * 128],
                                    hst_d[i * 128:(i + 1) * 128, :])
            # fp8 projection weights first: stage_a2(0) needs them
            wip = []
            for i2 in range(3):
                t = wpool.tile([128, 2 * 2 * HID], f8, tag=f"wip{i2}")
                nc.gpsimd.dma_start(
                    t[:].rearrange("f (two n) -> f two n", two=2), wip_d[i2])
                wip.append(t)
            biases = {}
            for nm, d, w in (("bq", bq_d, HSP), ("bk", bk_d, HSP),
                             ("bv", bv_d, HSP), ("bpv", bpv_d, DH),
                             ("bmlp", bmlp_d, HID)):
                t = cpool.tile([1, w], f32r, tag=nm)
                nc.scalar.dma_start(t[:], d)
                biases[nm] = t

            score_all = cpool.tile([S, H * QS], f32, tag="score_all")
            st_b = {}

            def stage_a1(qi):
                ip8 = iopool.tile([S, HID], f8, tag="ip8")
                nc.scalar.dma_start(ip8[:], ip_d[qi])
                st_b[('a', qi)] = ip8

            def stage_a2(qi):
                ip8 = st_b.pop(('a', qi))
                pr = ppool.tile([S, 2 * HID], f32, tag="proj")
                for i2 in range(3):
                    lhs = ip8[:, 256 * i2:256 * (i2 + 1)].rearrange(
                        "f (two c) -> f two c", two=2)
                    st = (i2 == 0)
                    sp = (i2 == 2)
                    wr = wip[i2][:].rearrange("f (two n) -> f two n", two=2)
                    for nch in range(3):
                        nc.tensor.matmul(pr[:, 512 * nch:512 * (nch + 1)],
                                         lhs, wr[:, :, 512 * nch:512 * (nch + 1)],
                                         start=st, stop=sp, perf_mode=DR)
                proj_sb = mid.tile([S, 2 * HID], rt, tag="proj_sb")
                nc.scalar.copy(proj_sb[:], pr[:])
                # pab write on the scalar HWDGE queue: gpsimd's SWDGE queue is
                # saturated by the strided q-broadcast packets
                pab = dpool.tile([S, 2 * HID], rt, tag="pab")
                nc.scalar.dma_start(pab[:], proj_sb[:])
                st_b[qi] = pab

            def stage_b(qi):
                # qe/ke land in separate contiguous halves of qk_t so the
                # q-broadcast is a contiguous 128-packet DMA (the strided
                # variant was 1536 packets and saturated the SWDGE queue);
                # the scrambled read splits into two reads on two queues.
                pab = st_b.pop(qi)
                qk_t = bpool.tile([S, 2 * HID], rt, tag="qk_t")
                src2 = pab[:].flatten().rearrange(
                    "(j k two d) -> two k j d", j=H, k=S, two=2, d=DH)
                qe_v = qk_t[:, 0:HID].rearrange("k (j d) -> k j d", j=H, d=DH)
                ke_v = qk_t[:, HID:2 * HID].rearrange("k (j d) -> k j d",
                                                      j=H, d=DH)
                # both scrambled reads on the sync HWDGE queue: the gpsimd
                # SWDGE engine was the per-block serializer (~3.6us issuing
                # the ke read + broadcast); it now carries only the broadcast
                nc.sync.dma_start(qe_v, src2[0])
                nc.sync.dma_start(ke_v, src2[1])
                qsrc = qt2_dram[qi:qi + 1, :].flatten().rearrange(
                    "(o j d) -> o j d", o=1, j=H, d=DH).broadcast_to([S, H, DH])
                nc.gpsimd.dma_start(qe_v, qsrc, accum_op=OP.add)
                hl0 = (12 * qi) // 128
                jsplit = min(12, 128 * (hl0 + 1) - 12 * qi)
                ranges = (((0, jsplit, hl0),) if jsplit >= 12 else
                          ((0, jsplit, hl0), (jsplit, 12, hl0 + 1)))
                for (jlo, jhi, hl) in ranges:
                    cnt = jhi - jlo
                    nc.vector.tensor_add(
                        ke_v[:, jlo:jhi, :], ke_v[:, jlo:jhi, :],
                        kh_sb[:, hl * DH:(hl + 1) * DH].rearrange(
                            "k (o d) -> k o d", o=1).broadcast_to([S, cnt, DH]))
                prod = bpool.tile([S, HID], rt, tag="prod")
                nc.vector.tensor_mul(
                    prod[:].rearrange("p (j d) -> p j d", j=H), qe_v, ke_v)
                nc.vector.tensor_reduce(
                    score_all[:, 12 * qi:12 * qi + 12],
                    prod[:].rearrange("p (j d) -> p j d", j=H),
                    axis=AX.X, op=OP.add)

            # deeper pipeline: the per-block chain (proj copy -> pab write ->
            # scrambled reads -> folds) is latency-bound (~3 serial DMA fixed
            # costs); 4 blocks in flight hides most of it
            LAG = 4
            PRE = 3
            for qi in range(PRE):
                stage_a1(qi)
            wqkv = {}
            for nm, d in (("wq", wq_d), ("wk", wk_d), ("wv", wv_d)):
                ch = []
                for i in range(6):
                    t = wpool.tile([128, HSP], f32r, tag=f"{nm}{i}")
                    nc.gpsimd.dma_start(t[:], d[i * 128:(i + 1) * 128, :])
                    ch.append(t)
                wqkv[nm] = ch
            wpv = []
            for i in range(6):
                t = wpool.tile([128, DH], f32r, tag=f"wpv{i}")
                nc.gpsimd.dma_start(t[:], wpv_d[i * 128:(i + 1) * 128, :])
                wpv.append(t)
            wma = wpool.tile([128, HID], f32r, tag="wma")
            nc.gpsimd.dma_start(wma[:], wma_d)
            wmb = wpool.tile([128, HID], f32r, tag="wmb")
            nc.gpsimd.dma_start(wmb[:], wmb_d)

            stage_a2(0)

            # ---- phase Q: 3-head q/k/v + pv projections ----
            def head_proj(wch, bias):
                ps = tpool.tile([S, 384], f32, tag="tp")
                for i in range(6):
                    nc.tensor.matmul(ps[:, 0:HSP],
                                     hst[:, i * 128:(i + 1) * 128],
                                     wch[i][:], start=(i == 0), stop=False)
                nc.tensor.matmul(ps[:, 0:HSP], onesr[:, 0:128], bias[:],
                                 start=False, stop=True)
                return ps

            ps = head_proj(wqkv["wk"], biases["bk"])
            kh_sb = cpool.tile([S, HS], rt, tag="kh_sb")
            nc.scalar.copy(kh_sb[:], ps[:, 0:HS])
            ps = head_proj(wqkv["wv"], biases["bv"])
            vh_sb = cpool.tile([S, HS], f32, tag="vh_sb")
            nc.vector.tensor_copy(vh_sb[:], ps[:, 0:HS])
            ps = head_proj(wqkv["wq"], biases["bq"])
            qh_sb = cpool.tile([S, HS], rt, tag="qh_sb")
            nc.scalar.copy(qh_sb[:], ps[:, 0:HS])
            # qt2[qi, j*64+d] = Qh[q', hl*64+d], (hl,q') = divmod(12*qi+j, 128)
            qt2_dram = dpool1.tile([QS, HID], rt, tag="qt2")
            nc.sync.dma_start(
                qt2_dram[:].flatten().rearrange("(h q d) -> q h d",
                                                h=NH, q=S, d=DH),
                qh_sb[:].rearrange("q (h d) -> q h d", h=NH, d=DH))

            ps = tpool.tile([S, 384], f32, tag="tp")
            for i in range(6):
                nc.tensor.matmul(ps[:, 0:DH], hst[:, i * 128:(i + 1) * 128],
                                 wpv[i][:], start=(i == 0), stop=False)
            nc.tensor.matmul(ps[:, 0:DH], onesr[:, 0:128], biases["bpv"][:],
                             start=False, stop=True)
            pv_sb = cpool.tile([S, DH], f32, tag="pv_sb")
            nc.scalar.copy(pv_sb[:], ps[:, 0:DH])

            # ---- softmax pieces (split: first half mid-loop) ----
            expall = cpool.tile([S, H * QS], f32, tag="expall")
            pden = tpool.tile([S, 384], f32, tag="tpden")
            SSCALE = 1.0 / (WSC * WSC * math.sqrt(DH))
            HALF = H * QS // 2   # 192 columns = blocks 0..15

            stage_a2(1)
            for qi in range(2, QS):
                if qi + 1 < QS:
                    stage_a1(qi + 1)
                if qi >= LAG:
                    stage_b(qi - LAG)
                stage_a2(qi)
                if qi == 19:
                    # blocks 0..15 scored (stage_b(17) issued); softmax half 1
                    nc.scalar.activation(expall[:, 0:HALF], score_all[:, 0:HALF],
                                         ACTF.Exp, bias=mask[:, 0:1],
                                         scale=SSCALE)
                    nc.tensor.matmul(pden[0:1, 0:HALF], ones[:, 0:1],
                                     expall[:, 0:HALF], start=True, stop=True)
            for qi in range(QS - LAG, QS):
                stage_b(qi)

            # ---- batched softmax over k' (partitions), second half ----
            nc.scalar.activation(expall[:, HALF:], score_all[:, HALF:],
                                 ACTF.Exp, bias=mask[:, 0:1], scale=SSCALE)
            nc.tensor.matmul(pden[0:1, HALF:H * QS], ones[:, 0:1],
                             expall[:, HALF:], start=True, stop=True)
            den_sb = cpool.tile([1, H * QS], f32, tag="den_sb")
            nc.scalar.copy(den_sb[:], pden[0:1, 0:H * QS])
            pdenb = tpool.tile([S, 384], f32, tag="tp")
            nc.tensor.matmul(pdenb[:, 0:H * QS], onesr[:, 0:128].bitcast(f32),
                             den_sb[:], start=True, stop=True)
            recipb = cpool.tile([S, H * QS], f32, tag="recipb")
            nc.vector.reciprocal(recipb[:], pdenb[:, 0:H * QS])
            expn = cpool.tile([S, H * QS], f32, tag="expn")
            nc.vector.tensor_mul(expn[:], expall[:], recipb[:])

            # ---- ctx^T blocks + parse block: [128, 256] ----
            # chunk A (cols 0:128):  partitions 0:64 = hl0, 64:128 = hl1
            # chunk B (cols 128:256): partitions 0:64 = hl2, 64:128 = parse
            pctx = tpool.tile([S, 384], f32, tag="tp")
            for hl in range(NH):
                po = (hl % 2) * DH
                co = (hl // 2) * S
                nc.tensor.matmul(pctx[po:po + DH, co:co + S],
                                 vh_sb[:, hl * DH:(hl + 1) * DH],
                                 expn[:, hl * S:(hl + 1) * S],
                                 start=True, stop=True)
            nc.tensor.matmul(pctx[DH:2 * DH, S:2 * S], pv_sb[:],
                             spant[:].bitcast(f32), start=True, stop=True)
            ctxt = cpool.tile([S, 2 * S], f32r, tag="ctxt")
            nc.scalar.copy(ctxt[:], pctx[:, 0:2 * S])

            # ---- partial mlp: out[q', o] = ctxA.T@wmlpA + ctxB.T@wmlpB ----
            pout = ppool.tile([S, 2 * HID], f32, tag="proj")
            nc.tensor.matmul(pout[:, 0:512], ctxt[:, 0:S], wma[:, 0:512],
                             start=True, stop=False)
            nc.tensor.matmul(pout[:, 512:768], ctxt[:, 0:S], wma[:, 512:768],
                             start=True, stop=False)
            nc.tensor.matmul(pout[:, 0:512], ctxt[:, S:2 * S], wmb[:, 0:512],
                             start=False, stop=False)
            nc.tensor.matmul(pout[:, 512:768], ctxt[:, S:2 * S], wmb[:, 512:768],
                             start=False, stop=False)
            nc.tensor.matmul(pout[:, 0:512], onesr[:, 0:128],
                             biases["bmlp"][:, 0:512], start=False, stop=False)
            nc.tensor.matmul(pout[:, 512:768], onesr[:, 0:128],
                             biases["bmlp"][:, 512:768], start=False, stop=True)
            out_sb = cpool.tile([S, HID], f32, tag="out_sb")
            nc.vector.tensor_copy(out_sb[:], pout[:, 0:768])
            nc.sync.dma_start(out_d, out_sb[:])

    nc.compile()
    return nc


def _make_in_maps(inputs):
    import ml_dtypes
    f8 = ml_dtypes.float8_e4m3fn
    a = {k: np.ascontiguousarray(np.asarray(v, dtype=np.float32))
         for k, v in inputs.items()}
    zpad = np.zeros((HID, HSP - HS), np.float32)

    # Wip: x16, columns permuted to (m_hi, half, d), fp8, grouped as
    # [3][128][two][1536] (k-chunk pairs for DoubleRow)
    perm = np.array([hf * 768 + m * 64 + d
                     for m in range(12) for hf in range(2) for d in range(64)])
    wp = (a["Wip"] * WSC)[:, perm].reshape(6, 128, 2 * HID)
    wip8 = np.ascontiguousarray(
        wp.reshape(3, 2, 128, 2 * HID).transpose(0, 2, 1, 3)).astype(f8)

    in_maps = []
    for c in range(N_CORES):
        b = (c * QS) // S
        q0 = (c * QS) % S
        c4 = c % 4
        h0 = NH * c4
        wq_s = np.concatenate(
            [a["Wq"][:, h0 * DH:(h0 + NH) * DH] * WSC, zpad], 1)
        wk_s = np.concatenate(
            [a["Wk"][:, h0 * DH:(h0 + NH) * DH] * WSC, zpad], 1)
        wv_s = np.concatenate([a["Wv"][:, h0 * DH:(h0 + NH) * DH], zpad], 1)
        bpad = np.zeros(HSP - HS, np.float32)
        bq_s = np.concatenate([a["bq"][h0 * DH:(h0 + NH) * DH] * WSC, bpad])
        bk_s = np.concatenate([a["bk"][h0 * DH:(h0 + NH) * DH] * WSC, bpad])
        bv_s = np.concatenate([a["bv"][h0 * DH:(h0 + NH) * DH], bpad])
        spant = (a["span_mask"][b, 0].T if c4 == 0
                 else np.zeros((S, S), np.float32))
        bmlp = a["bmlp"] if c4 == 0 else np.zeros(HID, np.float32)
        wmlpa = a["Wmlp"][HS * c4:HS * c4 + 128]
        wmlpb = np.concatenate([a["Wmlp"][HS * c4 + 128:HS * (c4 + 1)],
                                a["Wmlp"][HID:HID + DH]], 0)
        # ip: pre-transposed per block: ipt[qi, p, i*128+s] = ip[qi, s, i*128+p]
        ipt8 = np.ascontiguousarray(
            a["inference_path"][b, q0:q0 + QS].reshape(QS, S, 6, 128)
            .transpose(0, 3, 2, 1)).astype(f8).reshape(QS, 128, HID)
        in_maps.append({
            "ip": ipt8,
            "hst": np.ascontiguousarray(a["hidden_states"][b].T),
            "mask": a["attention_mask"][b, 0, 0].reshape(S, 1),
            "spant": np.ascontiguousarray(spant),
            "wq": wq_s, "wk": wk_s, "wv": wv_s,
            "wpv": a["Wpv"], "wip": wip8,
            "wmlpa": np.ascontiguousarray(wmlpa),
            "wmlpb": np.ascontiguousarray(wmlpb),
            "bq": bq_s.reshape(1, HSP), "bk": bk_s.reshape(1, HSP),
            "bv": bv_s.reshape(1, HSP),
            "bpv": a["bpv"].reshape(1, DH),
            "bmlp": bmlp.reshape(1, HID),
            "ones": np.ones((S, 1), np.float32),
            "onesr": np.ones((1, S), np.float32),
        })
    return in_maps


def kernel(**inputs):
    if "nc" not in _CACHE:
        _CACHE["nc"] = _build_program()
    nc = _CACHE["nc"]
    in_maps = _make_in_maps(inputs)

    from concourse import bass_utils
    res = bass_utils.run_bass_kernel_spmd(nc, in_maps, core_ids=list(range(N_CORES)))
    out = np.zeros((B, S, HID), np.float32)
    for c in range(N_CORES):
        b = (c * QS) // S
        out[b] += res.results[c]["out"]
    return out


# revision 38
# speedup vs baseline: 1.0050x; 1.0050x over previous
"""BertSelfAttention with relative inference-path bias — Bass/Tile TRN2 kernel.

Shapes: B=2, S=128, H=12, DH=64, HID=768.  8 NeuronCores.

The reference materializes ip = inference_path @ Wip ([B,S,S,1536], 201MB)
and 5D ra/rb tensors; it reinterprets ra/rb via a RAW flat view
(`.reshape(B,H,S,S,DH)` on a [B,S,S,HID] array — torch .view semantics),
which scrambles cells: score pair w = (h,q') with w = h*128+q' draws its
bias row from raw cells t = 12*k_raw + c_idx of projection block
q_raw = w // 12, where (j, k') = divmod(t, 128), j = w - 12*q_raw.

Sharding: (b, q_raw) blocks, 32 per core; each core covers 3 heads x all
128 queries of its batch and emits a PARTIAL mlp output summed on the
host across the 4 cores of each batch.

v1: fp8(e4m3) DoubleRow projection.  The host pre-transposes + casts ip
to fp8 (so no on-chip transpose) and stores Wip * 16 in fp8 with columns
permuted to (m_hi, half, d) so the PSUM projection comes out with the
a/b halves interleaved at 64-elem granularity.  The DRAM round-trip then
needs ONE scrambled read per block (256B elements instead of 2x128B).
Wq/Wk/bq/bk are scaled x16 to match (scores come out x256; folded into
the softmax exp scale).
"""
import math
import sys

import numpy as np

if '/opt/trn_rl_repo' not in sys.path:
    sys.path.insert(0, '/opt/trn_rl_repo')

H = 12
DH = 64
HID = 768
B = 2
S = 128
N_CORES = 8
QS = (B * S) // N_CORES   # 32 q_raw blocks per core
NH = 3                    # heads per core
HS = NH * DH              # 192
HSP = 256                 # padded head-slice width (fp32r needs N>=256)
WSC = 16.0                # fp8 scale on Wip/Wq/Wk (scores x 16*16)

_CACHE = {}


def _build_program():
    import concourse.bass as bass
    import concourse.bacc as bacc
    import concourse.mybir as mybir
    import concourse.tile as tile

    f32 = mybir.dt.float32
    f32r = mybir.dt.float32r
    bf16 = mybir.dt.bfloat16
    f8 = mybir.dt.float8e4
    rt = bf16                  # round-trip dtype for scrambled tensors
    AX = mybir.AxisListType
    OP = mybir.AluOpType
    ACTF = mybir.ActivationFunctionType
    DR = mybir.MatmulPerfMode.DoubleRow

    nc = bacc.Bacc("TRN2", target_bir_lowering=False, debug=False,
                   num_devices=N_CORES)

    # ---- DRAM I/O (per-core shapes) ----
    ip_d = nc.dram_tensor("ip", [QS, S, HID], f8, kind="ExternalInput").ap()
    hst_d = nc.dram_tensor("hst", [HID, S], f32r, kind="ExternalInput").ap()
    mask_d = nc.dram_tensor("mask", [S, 1], f32, kind="ExternalInput").ap()
    spant_d = nc.dram_tensor("spant", [S, S], f32r, kind="ExternalInput").ap()
    wq_d = nc.dram_tensor("wq", [HID, HSP], f32r, kind="ExternalInput").ap()
    wk_d = nc.dram_tensor("wk", [HID, HSP], f32r, kind="ExternalInput").ap()
    wv_d = nc.dram_tensor("wv", [HID, HSP], f32r, kind="ExternalInput").ap()
    wpv_d = nc.dram_tensor("wpv", [HID, DH], f32r, kind="ExternalInput").ap()
    # wip: [3][128][two=2][2*HID] fp8, columns permuted (m_hi, half, d), x16
    wip_d = nc.dram_tensor("wip", [3, 128, 2, 2 * HID], f8,
                           kind="ExternalInput").ap()
    wma_d = nc.dram_tensor("wmlpa", [128, HID], f32r, kind="ExternalInput").ap()
    wmb_d = nc.dram_tensor("wmlpb", [128, HID], f32r, kind="ExternalInput").ap()
    bq_d = nc.dram_tensor("bq", [1, HSP], f32r, kind="ExternalInput").ap()
    bk_d = nc.dram_tensor("bk", [1, HSP], f32r, kind="ExternalInput").ap()
    bv_d = nc.dram_tensor("bv", [1, HSP], f32r, kind="ExternalInput").ap()
    bpv_d = nc.dram_tensor("bpv", [1, DH], f32r, kind="ExternalInput").ap()
    bmlp_d = nc.dram_tensor("bmlp", [1, HID], f32r, kind="ExternalInput").ap()
    ones_d = nc.dram_tensor("ones", [S, 1], f32, kind="ExternalInput").ap()
    onesr_d = nc.dram_tensor("onesr", [1, S], f32r, kind="ExternalInput").ap()
    out_d = nc.dram_tensor("out", [S, HID], f32, kind="ExternalOutput").ap()

    with tile.TileContext(nc) as tc:
        with (
            tc.tile_pool(name="wpool", bufs=1) as wpool,
            tc.tile_pool(name="cpool", bufs=1) as cpool,
            tc.tile_pool(name="iopool", bufs=4) as iopool,
            tc.tile_pool(name="bpool", bufs=6) as bpool,
            tc.tile_pool(name="mid", bufs=4) as mid,
            tc.tile_pool(name="ppool", bufs=2, space=bass.MemorySpace.PSUM) as ppool,
            tc.tile_pool(name="tpool", bufs=1, space=bass.MemorySpace.PSUM) as tpool,
            tc.tile_pool(name="dpool", bufs=6, space=bass.MemorySpace.DRAM) as dpool,
            tc.tile_pool(name="dpool1", bufs=1, space=bass.MemorySpace.DRAM) as dpool1,
        ):
            # ---- constants / weights to SBUF ----
            ones = cpool.tile([S, 1], f32, tag="ones")
            nc.scalar.dma_start(ones[:], ones_d)
            onesr = cpool.tile([1, S], f32r, tag="onesr")
            nc.scalar.dma_start(onesr[:], onesr_d)
            mask = cpool.tile([S, 1], f32, tag="mask")
            nc.scalar.dma_start(mask[:], mask_d)
            spant = cpool.tile([S, S], f32r, tag="spant")
            nc.scalar.dma_start(spant[:], spant_d)
            hst = cpool.tile([S, HID], f32r, tag="hst")
            for i in range(6):
                nc.gpsimd.dma_start(hst[:, i * 128:(i + 1) * 128],
                                    hst_d[i * 128:(i + 1) * 128, :])
            # fp8 projection weights first: stage_a2(0) needs them
            wip = []
            for i2 in range(3):
                t = wpool.tile([128, 2 * 2 * HID], f8, tag=f"wip{i2}")
                nc.gpsimd.dma_start(
                    t[:].rearrange("f (two n) -> f two n", two=2), wip_d[i2])
                wip.append(t)
            biases = {}
            for nm, d, w in (("bq", bq_d, HSP), ("bk", bk_d, HSP),
                             ("bv", bv_d, HSP), ("bpv", bpv_d, DH),
                             ("bmlp", bmlp_d, HID)):
                t = cpool.tile([1, w], f32r, tag=nm)
                nc.scalar.dma_start(t[:], d)
                biases[nm] = t

            score_all = cpool.tile([S, H * QS], f32, tag="score_all")
            st_b = {}

            def stage_a1(qi):
                ip8 = iopool.tile([S, HID], f8, tag="ip8")
                nc.scalar.dma_start(ip8[:], ip_d[qi])
                st_b[('a', qi)] = ip8

            def stage_a2(qi):
                ip8 = st_b.pop(('a', qi))
                pr = ppool.tile([S, 2 * HID], f32, tag="proj")
                for i2 in range(3):
                    lhs = ip8[:, 256 * i2:256 * (i2 + 1)].rearrange(
                        "f (two c) -> f two c", two=2)
                    st = (i2 == 0)
                    sp = (i2 == 2)
                    wr = wip[i2][:].rearrange("f (two n) -> f two n", two=2)
                    for nch in range(3):
                        nc.tensor.matmul(pr[:, 512 * nch:512 * (nch + 1)],
                                         lhs, wr[:, :, 512 * nch:512 * (nch + 1)],
                                         start=st, stop=sp, perf_mode=DR)
                # split the PSUM->SBUF copy across scalar+vector: halves run
                # in parallel, freeing the PSUM bank (ppool bufs=2 gates the
                # PE) and starting the pab write ~0.7us earlier
                proj_sb = mid.tile([S, 2 * HID], rt, tag="proj_sb")
                nc.scalar.copy(proj_sb[:, 0:HID], pr[:, 0:HID])
                nc.vector.tensor_copy(proj_sb[:, HID:2 * HID],
                                      pr[:, HID:2 * HID])
                # pab write on the scalar HWDGE queue: gpsimd's SWDGE queue is
                # saturated by the strided q-broadcast packets
                pab = dpool.tile([S, 2 * HID], rt, tag="pab")
                nc.scalar.dma_start(pab[:], proj_sb[:])
                st_b[qi] = pab

            def stage_b(qi):
                # qe/ke land in separate contiguous halves of qk_t so the
                # q-broadcast is a contiguous 128-packet DMA (the strided
                # variant was 1536 packets and saturated the SWDGE queue);
                # the scrambled read splits into two reads on two queues.
                pab = st_b.pop(qi)
                qk_t = bpool.tile([S, 2 * HID], rt, tag="qk_t")
                src2 = pab[:].flatten().rearrange(
                    "(j k two d) -> two k j d", j=H, k=S, two=2, d=DH)
                qe_v = qk_t[:, 0:HID].rearrange("k (j d) -> k j d", j=H, d=DH)
                ke_v = qk_t[:, HID:2 * HID].rearrange("k (j d) -> k j d",
                                                      j=H, d=DH)
                nc.sync.dma_start(qe_v, src2[0])
                nc.gpsimd.dma_start(ke_v, src2[1])
                qsrc = qt2_dram[qi:qi + 1, :].flatten().rearrange(
                    "(o j d) -> o j d", o=1, j=H, d=DH).broadcast_to([S, H, DH])
                nc.gpsimd.dma_start(qe_v, qsrc, accum_op=OP.add)
                hl0 = (12 * qi) // 128
                jsplit = min(12, 128 * (hl0 + 1) - 12 * qi)
                ranges = (((0, jsplit, hl0),) if jsplit >= 12 else
                          ((0, jsplit, hl0), (jsplit, 12, hl0 + 1)))
                for (jlo, jhi, hl) in ranges:
                    cnt = jhi - jlo
                    nc.vector.tensor_add(
                        ke_v[:, jlo:jhi, :], ke_v[:, jlo:jhi, :],
                        kh_sb[:, hl * DH:(hl + 1) * DH].rearrange(
                            "k (o d) -> k o d", o=1).broadcast_to([S, cnt, DH]))
                prod = bpool.tile([S, HID], rt, tag="prod")
                nc.vector.tensor_mul(
                    prod[:].rearrange("p (j d) -> p j d", j=H), qe_v, ke_v)
                nc.vector.tensor_reduce(
                    score_all[:, 12 * qi:12 * qi + 12],
                    prod[:].rearrange("p (j d) -> p j d", j=H),
                    axis=AX.X, op=OP.add)

            # deeper pipeline: the per-block chain (proj copy -> pab write ->
            # scrambled reads -> folds) is latency-bound (~3 serial DMA fixed
            # costs); 4 blocks in flight hides most of it
            LAG = 4
            PRE = 3
            for qi in range(PRE):
                stage_a1(qi)
            wqkv = {}
            for nm, d in (("wq", wq_d), ("wk", wk_d), ("wv", wv_d)):
                ch = []
                for i in range(6):
                    t = wpool.tile([128, HSP], f32r, tag=f"{nm}{i}")
                    nc.gpsimd.dma_start(t[:], d[i * 128:(i + 1) * 128, :])
                    ch.append(t)
                wqkv[nm] = ch
            wpv = []
            for i in range(6):
                t = wpool.tile([128, DH], f32r, tag=f"wpv{i}")
                nc.gpsimd.dma_start(t[:], wpv_d[i * 128:(i + 1) * 128, :])
                wpv.append(t)
            wma = wpool.tile([128, HID], f32r, tag="wma")
            nc.gpsimd.dma_start(wma[:], wma_d)
            wmb = wpool.tile([128, HID], f32r, tag="wmb")
            nc.gpsimd.dma_start(wmb[:], wmb_d)

            stage_a2(0)

            # ---- phase Q: 3-head q/k/v + pv projections ----
            def head_proj(wch, bias):
                ps = tpool.tile([S, 384], f32, tag="tp")
                for i in range(6):
                    nc.tensor.matmul(ps[:, 0:HSP],
                                     hst[:, i * 128:(i + 1) * 128],
                                     wch[i][:], start=(i == 0), stop=False)
                nc.tensor.matmul(ps[:, 0:HSP], onesr[:, 0:128], bias[:],
                                 start=False, stop=True)
                return ps

            ps = head_proj(wqkv["wk"], biases["bk"])
            kh_sb = cpool.tile([S, HS], rt, tag="kh_sb")
            nc.scalar.copy(kh_sb[:], ps[:, 0:HS])
            ps = head_proj(wqkv["wv"], biases["bv"])
            vh_sb = cpool.tile([S, HS], f32, tag="vh_sb")
            nc.vector.tensor_copy(vh_sb[:], ps[:, 0:HS])
            ps = head_proj(wqkv["wq"], biases["bq"])
            qh_sb = cpool.tile([S, HS], rt, tag="qh_sb")
            nc.scalar.copy(qh_sb[:], ps[:, 0:HS])
            # qt2[qi, j*64+d] = Qh[q', hl*64+d], (hl,q') = divmod(12*qi+j, 128)
            qt2_dram = dpool1.tile([QS, HID], rt, tag="qt2")
            nc.sync.dma_start(
                qt2_dram[:].flatten().rearrange("(h q d) -> q h d",
                                                h=NH, q=S, d=DH),
                qh_sb[:].rearrange("q (h d) -> q h d", h=NH, d=DH))

            ps = tpool.tile([S, 384], f32, tag="tp")
            for i in range(6):
                nc.tensor.matmul(ps[:, 0:DH], hst[:, i * 128:(i + 1) * 128],
                                 wpv[i][:], start=(i == 0), stop=False)
            nc.tensor.matmul(ps[:, 0:DH], onesr[:, 0:128], biases["bpv"][:],
                             start=False, stop=True)
            pv_sb = cpool.tile([S, DH], f32, tag="pv_sb")
            nc.scalar.copy(pv_sb[:], ps[:, 0:DH])

            # ---- softmax pieces (split: first half mid-loop) ----
            expall = cpool.tile([S, H * QS], f32, tag="expall")
            pden = tpool.tile([S, 384], f32, tag="tpden")
            SSCALE = 1.0 / (WSC * WSC * math.sqrt(DH))
            HALF = H * QS // 2   # 192 columns = blocks 0..15

            stage_a2(1)
            for qi in range(2, QS):
                if qi + 1 < QS:
                    stage_a1(qi + 1)
                if qi >= LAG:
                    stage_b(qi - LAG)
                stage_a2(qi)
                if qi == 19:
                    # blocks 0..15 scored (stage_b(17) issued); softmax half 1
                    nc.scalar.activation(expall[:, 0:HALF], score_all[:, 0:HALF],
                                         ACTF.Exp, bias=mask[:, 0:1],
                                         scale=SSCALE)
                    nc.tensor.matmul(pden[0:1, 0:HALF], ones[:, 0:1],
                                     expall[:, 0:HALF], start=True, stop=True)
            for qi in range(QS - LAG, QS):
                stage_b(qi)

            # ---- batched softmax over k' (partitions), second half ----
            nc.scalar.activation(expall[:, HALF:], score_all[:, HALF:],
                                 ACTF.Exp, bias=mask[:, 0:1], scale=SSCALE)
            nc.tensor.matmul(pden[0:1, HALF:H * QS], ones[:, 0:1],
                             expall[:, HALF:], start=True, stop=True)
            den_sb = cpool.tile([1, H * QS], f32, tag="den_sb")
            nc.scalar.copy(den_sb[:], pden[0:1, 0:H * QS])
            pdenb = tpool.tile([S, 384], f32, tag="tp")
            nc.tensor.matmul(pdenb[:, 0:H * QS], onesr[:, 0:128].bitcast(f32),
                             den_sb[:], start=True, stop=True)
            recipb = cpool.tile([S, H * QS], f32, tag="recipb")
            nc.vector.reciprocal(recipb[:], pdenb[:, 0:H * QS])
            expn = cpool.tile([S, H * QS], f32, tag="expn")
            nc.vector.tensor_mul(expn[:], expall[:], recipb[:])

            # ---- ctx^T blocks + parse block: [128, 256] ----
            # chunk A (cols 0:128):  partitions 0:64 = hl0, 64:128 = hl1
            # chunk B (cols 128:256): partitions 0:64 = hl2, 64:128 = parse
            pctx = tpool.tile([S, 384], f32, tag="tp")
            for hl in range(NH):
                po = (hl % 2) * DH
                co = (hl // 2) * S
                nc.tensor.matmul(pctx[po:po + DH, co:co + S],
                                 vh_sb[:, hl * DH:(hl + 1) * DH],
                                 expn[:, hl * S:(hl + 1) * S],
                                 start=True, stop=True)
            nc.tensor.matmul(pctx[DH:2 * DH, S:2 * S], pv_sb[:],
                             spant[:].bitcast(f32), start=True, stop=True)
            ctxt = cpool.tile([S, 2 * S], f32r, tag="ctxt")
            nc.scalar.copy(ctxt[:], pctx[:, 0:2 * S])

            # ---- partial mlp: out[q', o] = ctxA.T@wmlpA + ctxB.T@wmlpB ----
            pout = ppool.tile([S, 2 * HID], f32, tag="proj")
            nc.tensor.matmul(pout[:, 0:512], ctxt[:, 0:S], wma[:, 0:512],
                             start=True, stop=False)
            nc.tensor.matmul(pout[:, 512:768], ctxt[:, 0:S], wma[:, 512:768],
                             start=True, stop=False)
            nc.tensor.matmul(pout[:, 0:512], ctxt[:, S:2 * S], wmb[:, 0:512],
                             start=False, stop=False)
            nc.tensor.matmul(pout[:, 512:768], ctxt[:, S:2 * S], wmb[:, 512:768],
                             start=False, stop=False)
            nc.tensor.matmul(pout[:, 0:512], onesr[:, 0:128],
                             biases["bmlp"][:, 0:512], start=False, stop=False)
            nc.tensor.matmul(pout[:, 512:768], onesr[:, 0:128],
                             biases["bmlp"][:, 512:768], start=False, stop=True)
            out_sb = cpool.tile([S, HID], f32, tag="out_sb")
            nc.vector.tensor_copy(out_sb[:], pout[:, 0:768])
            nc.sync.dma_start(out_d, out_sb[:])

    nc.compile()
    return nc


def _make_in_maps(inputs):
    import ml_dtypes
    f8 = ml_dtypes.float8_e4m3fn
    a = {k: np.ascontiguousarray(np.asarray(v, dtype=np.float32))
         for k, v in inputs.items()}
    zpad = np.zeros((HID, HSP - HS), np.float32)

    # Wip: x16, columns permuted to (m_hi, half, d), fp8, grouped as
    # [3][128][two][1536] (k-chunk pairs for DoubleRow)
    perm = np.array([hf * 768 + m * 64 + d
                     for m in range(12) for hf in range(2) for d in range(64)])
    wp = (a["Wip"] * WSC)[:, perm].reshape(6, 128, 2 * HID)
    wip8 = np.ascontiguousarray(
        wp.reshape(3, 2, 128, 2 * HID).transpose(0, 2, 1, 3)).astype(f8)

    in_maps = []
    for c in range(N_CORES):
        b = (c * QS) // S
        q0 = (c * QS) % S
        c4 = c % 4
        h0 = NH * c4
        wq_s = np.concatenate(
            [a["Wq"][:, h0 * DH:(h0 + NH) * DH] * WSC, zpad], 1)
        wk_s = np.concatenate(
            [a["Wk"][:, h0 * DH:(h0 + NH) * DH] * WSC, zpad], 1)
        wv_s = np.concatenate([a["Wv"][:, h0 * DH:(h0 + NH) * DH], zpad], 1)
        bpad = np.zeros(HSP - HS, np.float32)
        bq_s = np.concatenate([a["bq"][h0 * DH:(h0 + NH) * DH] * WSC, bpad])
        bk_s = np.concatenate([a["bk"][h0 * DH:(h0 + NH) * DH] * WSC, bpad])
        bv_s = np.concatenate([a["bv"][h0 * DH:(h0 + NH) * DH], bpad])
        spant = (a["span_mask"][b, 0].T if c4 == 0
                 else np.zeros((S, S), np.float32))
        bmlp = a["bmlp"] if c4 == 0 else np.zeros(HID, np.float32)
        wmlpa = a["Wmlp"][HS * c4:HS * c4 + 128]
        wmlpb = np.concatenate([a["Wmlp"][HS * c4 + 128:HS * (c4 + 1)],
                                a["Wmlp"][HID:HID + DH]], 0)
        # ip: pre-transposed per block: ipt[qi, p, i*128+s] = ip[qi, s, i*128+p]
        ipt8 = np.ascontiguousarray(
            a["inference_path"][b, q0:q0 + QS].reshape(QS, S, 6, 128)
            .transpose(0, 3, 2, 1)).astype(f8).reshape(QS, 128, HID)
        in_maps.append({
            "ip": ipt8,
            "hst": np.ascontiguousarray(a["hidden_states"][b].T),
            "mask": a["attention_mask"][b, 0, 0].reshape(S, 1),
            "spant": np.ascontiguousarray(spant),
            "wq": wq_s, "wk": wk_s, "wv": wv_s,
            "wpv": a["Wpv"], "wip": wip8,
            "wmlpa": np.ascontiguousarray(wmlpa),
            "wmlpb": np.ascontiguousarray(wmlpb),
            "bq": bq_s.reshape(1, HSP), "bk": bk_s.reshape(1, HSP),
            "bv": bv_s.reshape(1, HSP),
            "bpv": a["bpv"].reshape(1, DH),
            "bmlp": bmlp.reshape(1, HID),
            "ones": np.ones((S, 1), np.float32),
            "onesr": np.ones((1, S), np.float32),
        })
    return in_maps


def kernel(**inputs):
    if "nc" not in _CACHE:
        _CACHE["nc"] = _build_program()
    nc = _CACHE["nc"]
    in_maps = _make_in_maps(inputs)

    from concourse import bass_utils
    res = bass_utils.run_bass_kernel_spmd(nc, in_maps, core_ids=list(range(N_CORES)))
    out = np.zeros((B, S, HID), np.float32)
    for c in range(N_CORES):
        b = (c * QS) // S
        out[b] += res.results[c]["out"]
    return out
